# revision 23
# baseline (speedup 1.0000x reference)
"""Muskingum-Cunge river routing (depth-13 binary tree, N=8191, T=2048) on
8 Trainium2 NeuronCores — parallel-in-time Picard solver, v3.

Per reach, the MC update O_t = C1 I_t + C2 I_{t-1} + C3 O_{t-1} is a linear
recurrence once the (flow-dependent) coefficients are frozen; each Picard
pass recomputes the coefficients from the previous trajectory and re-solves
the recurrence with one DVE tensor_tensor_scan along time (the scan is
serial along the free dim, ~9 cycles/column, so scan instances are the
budget that matters). The relu clamp is folded into the fixed point via
B = (D + r) - q2*((K/dt)D + (r - I_old)) with r = relu(-U_prev).

Structure per core (core c owns the subtree under the c-th level-3 node):
  - levels 12..10 (512/256/128 nodes): node-major [128, T] tiles,
    K=(1,2,2) Picard passes, exact child inputs.
  - levels 9..3 (127 nodes): ONE stacked [127, T] Jacobi-Picard system,
    S_STACK passes; parent<-children pair-sum via a 0/1 matmul on the idle
    PE. No chunked layout, no DRAM scatter/gather, no carry machinery.
  - AllGather of the 8 level-3 root trajectories (64 KB), then the 7-node
    top tree (levels 2..0) as another stacked system, S_TOP passes.
All lateral inflows arrive in one partition-major [128, ~18.4k] block so
the whole 8.4 MB loads with three fat DMAs (384 descriptors, each under
the 64 KB descriptor limit) instead of ~1100 row-tile descriptors.

kernel() runs the split p1/p2 programs (host-gathered roots, no on-device
collective); the timed path uses part="all" (one NEFF per execution, the
AllGather in-NEFF) because every extra NEFF launch in a pipelined chain
costs ~3 ms on this runtime.
"""
import os
import sys

import numpy as np

for _p in ("/opt/trn_rl_repo", "/root/.axon_site/_ro/trn_rl_repo"):
    if os.path.isdir(_p) and _p not in sys.path:
        sys.path.insert(0, _p)

DEPTH = 13
N = 2**DEPTH - 1
T = 2048
NC = 8
F32 = np.float32

K_BIG = (1, 2, 2)  # Picard passes for levels 12, 11, 10
S_STACK = 8        # joint Jacobi-Picard passes for stacked levels 9..3
S_TOP = 4          # joint passes for the top tree (levels 2..0)

BIG_LEVELS = [(12, 512), (11, 256), (10, 128)]
K_OF_LEVEL = {12: K_BIG[0], 11: K_BIG[1], 10: K_BIG[2]}
STACK_LEVELS = [9, 8, 7, 6, 5, 4, 3]           # 64+32+16+8+4+2+1 = 127 rows
R_STACK = 127
R_TOP = 7
NBIG_TILES = 4 + 2 + 1
NGRP = NBIG_TILES + 2                          # const col groups: 7 big + stack + top
NCOL = NGRP * 4 + 1
_LN2DT_COL = NGRP * 4
AMAT_COLS = R_STACK + R_TOP                    # stack lhsT (127) + top lhsT (7)

# LAT2 layout: 7 big-tile blocks of 2049 cols (col 0 zero), then the stack
# block (2048 cols, rows 0..126), then the top block (2048 cols, rows 0..6).
_BIG_OFF = [g * 2049 for g in range(NBIG_TILES)]
_STACK_OFF = NBIG_TILES * 2049
_TOP_OFF = _STACK_OFF + T
LAT2_COLS = _TOP_OFF + T


def _build_ord():
    ORD = [np.array([0], dtype=np.int64)]
    for l in range(DEPTH - 1):
        cur = ORD[l]
        nxt = np.empty(2 * len(cur), dtype=np.int64)
        nxt[: len(cur)] = 2 * cur + 1
        nxt[len(cur):] = 2 * cur + 2
        ORD.append(nxt)
    return ORD


ORD = _build_ord()


def _level_nodes(core, lv):
    return ORD[lv] if lv < 3 else ORD[lv][core::NC]


# stack row base per level (level 9 first)
_STACK_BASE = {}
_off = 0
for _lv in STACK_LEVELS:
    _STACK_BASE[_lv] = _off
    _off += len(ORD[_lv]) // NC


def _stack_amat_T():
    """[127, 127] lhsT: A_T[child_row, parent_row] = 1 (in-stack coupling)."""
    A = np.zeros((R_STACK, R_STACK), F32)
    for lv in STACK_LEVELS[1:]:                 # parents: levels 8..3
        nl = len(ORD[lv]) // NC
        bp = _STACK_BASE[lv]
        bc = _STACK_BASE[lv + 1]
        for j in range(nl):
            A[bc + j, bp + j] = 1.0
            A[bc + j + nl, bp + j] = 1.0
    return A


def _top_amat_T():
    """[7, 7] lhsT for top rows: 0..3 = level-2 canon (ORD[2]), 4..5 =
    level-1 canon (ORD[1]), 6 = root."""
    A = np.zeros((R_TOP, R_TOP), F32)
    # level-1 canon j (rows 4,5): children = level-2 canon j and j+2
    A[0, 4] = A[2, 4] = 1.0
    A[1, 5] = A[3, 5] = 1.0
    # root (row 6): children = level-1 canon 0,1
    A[4, 6] = A[5, 6] = 1.0
    return A


def _host_precompute(inputs):
    lat = np.ascontiguousarray(np.asarray(inputs["lateral_inflows"], F32))
    n_ = np.asarray(inputs["manning_n"], F32).astype(np.float64)
    L = np.asarray(inputs["lengths"], F32).astype(np.float64)
    S = np.asarray(inputs["slopes"], F32).astype(np.float64)
    wc = np.asarray(inputs["width_coefs"], F32).astype(np.float64)
    we = np.asarray(inputs["width_exps"], F32).astype(np.float64)
    dc = np.asarray(inputs["depth_coefs"], F32).astype(np.float64)
    de = np.asarray(inputs["depth_exps"], F32).astype(np.float64)
    c0 = (5.0 / 3.0) * dc ** (2.0 / 3.0) * np.sqrt(S) / n_
    a1n = -(2.0 / 3.0) * de
    a3 = 1.0 - we - (2.0 / 3.0) * de
    ln_half = np.log(0.5)
    P4 = np.log(L / c0) + a1n * ln_half
    P3 = np.log(0.5 / (wc * S * L * c0)) + a3 * ln_half + np.log(2.0)
    consts = np.stack([a1n, a3, P4, P3]).astype(F32)      # [4, N]

    amat = np.zeros((128, AMAT_COLS), F32)
    amat[:R_STACK, :R_STACK] = _stack_amat_T()
    amat[:R_TOP, R_STACK:] = _top_amat_T()

    top_nodes = np.concatenate([ORD[2], ORD[1], ORD[0]])

    in_maps = []
    for core in range(NC):
        LAT2 = np.zeros((128, LAT2_COLS), F32)
        stack_nodes = np.concatenate(
            [_level_nodes(core, lv) for lv in STACK_LEVELS])
        g = 0
        for lv, n in BIG_LEVELS:
            nodes = _level_nodes(core, lv)
            for ti in range(n // 128):
                LAT2[:, _BIG_OFF[g] + 1:_BIG_OFF[g] + 1 + T] = \
                    lat[:, nodes[ti * 128:(ti + 1) * 128]].T
                g += 1
        LAT2[:R_STACK, _STACK_OFF:_STACK_OFF + T] = lat[:, stack_nodes].T
        LAT2[:R_TOP, _TOP_OFF:_TOP_OFF + T] = lat[:, top_nodes].T

        ccols = []
        for lv, n in BIG_LEVELS:
            nodes = _level_nodes(core, lv)
            for ti in range(n // 128):
                ccols.append(consts[:, nodes[ti * 128:(ti + 1) * 128]])
        cs = np.zeros((4, 128), F32)
        cs[:, :R_STACK] = consts[:, stack_nodes]
        ccols.append(cs)
        ct = np.zeros((4, 128), F32)
        ct[:, :R_TOP] = consts[:, top_nodes]
        ccols.append(ct)
        ccols.append(np.full((1, 128), np.log(2.0 * float(inputs["dt"])), F32))
        CST = np.ascontiguousarray(np.concatenate(ccols, axis=0).T.astype(F32))
        in_maps.append({"lat": np.ascontiguousarray(LAT2), "cst": CST,
                        "amat": amat})
    return in_maps


def _build_bass(dtf, part="all", stop_after=None):
    """Build the bass program.

    part="all": the whole routing step in one program, with the in-NEFF
        AllGather of root trajectories (used by the timed path: one NEFF
        launch per execution).
    part="p1": big + stacked levels only; outputs this core's level-3 root
        trajectory `root` [1, T]. No collectives.
    part="p2": inputs the 8 gathered root trajectories `roots` [8, T]
        (core/canon order); solves the top tree, outputs `out` [1, T].
    """
    from contextlib import ExitStack

    import concourse.bass as bass
    import concourse.tile as tile
    from concourse import bacc, mybir

    f32 = mybir.dt.float32
    OP = mybir.AluOpType
    AF = mybir.ActivationFunctionType
    inv_dt = 1.0 / dtf

    nc = bacc.Bacc("TRN2", target_bir_lowering=False, debug=False,
                   num_devices=NC)
    lat_d = nc.dram_tensor("lat", [128, LAT2_COLS], f32,
                           kind="ExternalInput").ap()
    cst_d = nc.dram_tensor("cst", [128, NCOL], f32, kind="ExternalInput").ap()
    amat_d = nc.dram_tensor("amat", [128, AMAT_COLS], f32,
                            kind="ExternalInput").ap()
    if part == "p1":
        out_d = nc.dram_tensor("root", [1, T], f32, kind="ExternalOutput").ap()
    elif part == "p2":
        roots_d = nc.dram_tensor("roots", [NC, T], f32,
                                 kind="ExternalInput").ap()
        out_d = nc.dram_tensor("out", [1, T], f32, kind="ExternalOutput").ap()
    else:
        out_d = nc.dram_tensor("out", [1, T], f32, kind="ExternalOutput").ap()

    with tile.TileContext(nc) as tc, ExitStack() as ctx:
        cpool = ctx.enter_context(tc.tile_pool(name="const", bufs=1))
        opool = ctx.enter_context(tc.tile_pool(name="lvlO", bufs=5))
        spool = ctx.enter_context(tc.tile_pool(name="scr", bufs=1))
        psum = ctx.enter_context(tc.tile_pool(name="ps", bufs=4, space="PSUM"))
        if part == "all":
            dram = ctx.enter_context(tc.tile_pool(name="dram", bufs=1,
                                                  space="DRAM"))

        cst = cpool.tile([128, NCOL], f32)
        nc.sync.dma_start(cst[:], cst_d)
        amat = cpool.tile([128, AMAT_COLS], f32)
        nc.sync.dma_start(amat[:], amat_d)

        if part in ("all", "p1"):
            LATALL = spool.tile([128, LAT2_COLS], f32, tag="LAT")
            # chunked: one descriptor per partition per chunk must stay
            # under the 64 KB DMA descriptor size limit
            third = 6150
            for c0 in range(0, LAT2_COLS, third):
                c1 = min(c0 + third, LAT2_COLS)
                nc.sync.dma_start(LATALL[:, c0:c1], lat_d[:, c0:c1])
        else:
            LATALL = spool.tile([128, T], f32, tag="LAT")
            nc.sync.dma_start(LATALL[:], lat_d[:, _TOP_OFF:_TOP_OFF + T])

        def cc(grp, R):
            c0 = grp * 4
            return (cst[0:R, c0:c0 + 1], cst[0:R, c0 + 1:c0 + 2],
                    cst[0:R, c0 + 2:c0 + 3], cst[0:R, c0 + 3:c0 + 4])

        def sc(tag, R):
            t = spool.tile([128, T], f32, tag=tag, name=f"scr_{tag}")
            return t[0:R, :]

        def emit_group(R, S, grp, Obuf, Ibase, icol, latE=None, lhsT=None):
            """Picard-solve one group of R reaches over the full horizon.

            The inflow lives in Ibase[0:R, icol:icol+T+1]; col icol must be
            zero. If latE is None the inflow is fixed (big levels); else each
            pass rebuilds it as latE + lhsT.T @ O and Obuf[:,1:] must start
            zeroed. Obuf: [128, T+1]; on return holds O with col 0 = 0.
            """
            a1, a3, P4, P3 = cc(grp, R)
            U = spool.tile([128, T + 1], f32, tag="gU")
            r = spool.tile([128, T + 1], f32, tag="gr")
            nc.vector.memset(r[0:R, :], 0.0)
            nc.vector.memset(Obuf[0:R, 0:1], 0.0)
            I1 = Ibase[0:R, icol + 1:icol + 1 + T]
            I0 = Ibase[0:R, icol:icol + T]
            for it in range(S):
                if latE is not None:
                    for j in range(4):
                        ps = psum.tile([128, 512], f32, tag=f"ps{j % 2}")
                        nc.tensor.matmul(ps[0:R, :], lhsT,
                                         Obuf[0:R, 1 + 512 * j:1 + 512 * (j + 1)],
                                         start=True, stop=True)
                        nc.vector.tensor_add(
                            Ibase[0:R, icol + 1 + 512 * j:icol + 1 + 512 * (j + 1)],
                            ps[0:R, :], latE[0:R, 512 * j:512 * (j + 1)])
                Oold = I0 if it == 0 else Obuf[0:R, 0:T]
                s1 = sc("s1", R)
                nc.gpsimd.tensor_add(s1, I1, Oold)
                s2 = sc("s2", R)
                nc.vector.tensor_scalar(s2, s1, 2e-3, None, op0=OP.max)
                lq = sc("s1", R)
                nc.scalar.activation(lq, s2, AF.Ln)
                K_ = sc("s3", R)
                nc.scalar.activation(K_, lq, AF.Exp, bias=P4, scale=a1)
                tt = sc("s2", R)
                nc.scalar.activation(tt, lq, AF.Exp, bias=P3, scale=a3)
                w1 = sc("s4", R)
                nc.vector.tensor_scalar(w1, tt, 1.0, None, op0=OP.min)
                v1 = sc("s2", R)
                nc.gpsimd.tensor_mul(v1, K_, w1)
                den = sc("s4", R)
                nc.vector.scalar_tensor_tensor(den, K_, dtf, v1, OP.add,
                                               OP.add)
                lnd = sc("s8", R)
                nc.scalar.activation(lnd, den, AF.Ln)
                q2 = sc("s2", R)
                nc.scalar.activation(q2, lnd, AF.Exp,
                                     bias=cst[0:R, _LN2DT_COL:_LN2DT_COL + 1],
                                     scale=-1.0)
                C3 = sc("s4", R)
                nc.vector.tensor_scalar(C3, q2, -1.0, 1.0, op0=OP.mult,
                                        op1=OP.add)
                D = sc("s8", R)
                nc.gpsimd.tensor_sub(D, I1, I0)
                E = sc("s5", R)
                nc.gpsimd.tensor_add(E, D, r[0:R, 0:T])
                G = sc("s6", R)
                nc.gpsimd.tensor_sub(G, r[0:R, 0:T], I0)
                h1 = sc("s7", R)
                nc.gpsimd.tensor_mul(h1, K_, D)
                H = sc("s3", R)
                nc.vector.scalar_tensor_tensor(H, h1, inv_dt, G, OP.mult,
                                               OP.add)
                zz = sc("s7", R)
                nc.vector.tensor_mul(zz, q2, H)
                B = sc("s1", R)
                nc.vector.tensor_sub(B, E, zz)
                nc.vector.tensor_tensor_scan(U[0:R, 1:], C3, B, 0.0,
                                             OP.mult, OP.add)
                nc.scalar.activation(Obuf[0:R, 1:], U[0:R, 1:], AF.Relu)
                if it < S - 1:
                    nc.scalar.activation(r[0:R, 1:], U[0:R, 1:], AF.Relu,
                                         scale=-1.0)

        Ost = None
        if part in ("all", "p1"):
            # ---- big levels (12, 11, 10): exact inputs, fixed inflow ----
            grp = 0
            child_tiles = None
            O10 = None
            for lv, n in BIG_LEVELS:
                ntile = n // 128
                tiles = []
                for ti in range(ntile):
                    off = _BIG_OFF[grp]
                    if lv != 12:
                        sup = sc("s6", 128)
                        nc.vector.tensor_add(sup, child_tiles[ti][:, 1:],
                                             child_tiles[ti + ntile][:, 1:])
                        nc.vector.tensor_add(
                            LATALL[:, off + 1:off + 1 + T],
                            LATALL[:, off + 1:off + 1 + T], sup)
                    Obig = opool.tile([128, T + 1], f32, tag="bigO")
                    emit_group(128, K_OF_LEVEL[lv], grp, Obig, LATALL[:], off)
                    tiles.append(Obig)
                    grp += 1
                child_tiles = tiles
                if lv == 10:
                    O10 = tiles[0]

            if stop_after == "big":
                nc.sync.dma_start(out_d, O10[0:1, 1:])
            else:
                # ---- stacked levels 9..3 ----
                tmp = sc("s5", 128)
                nc.sync.dma_start(tmp[0:64, :], O10[64:128, 1:])
                latE = LATALL[:, _STACK_OFF:_STACK_OFF + T]
                nc.vector.tensor_add(latE[0:64, :], latE[0:64, :],
                                     O10[0:64, 1:])
                nc.vector.tensor_add(latE[0:64, :], latE[0:64, :],
                                     tmp[0:64, :])

                Ist = spool.tile([128, T + 1], f32, tag="bI")
                nc.vector.memset(Ist[0:R_STACK, 0:1], 0.0)
                Ost = opool.tile([128, T + 1], f32, tag="bigO")
                nc.gpsimd.memset(Ost[0:R_STACK, 1:], 0.0)
                emit_group(R_STACK, S_STACK, NBIG_TILES, Ost, Ist[:], 0,
                           latE=latE, lhsT=amat[0:R_STACK, 0:R_STACK])
                if part == "p1":
                    nc.sync.dma_start(out_d, Ost[R_STACK - 1:R_STACK, 1:])

        if part == "all" and stop_after is None:
            b_in = dram.tile([1, T], f32)
            b_out = dram.tile([NC, T], f32)
            nc.sync.dma_start(b_in[:], Ost[R_STACK - 1:R_STACK, 1:])
            nc.gpsimd.collective_compute(
                "AllGather", OP.bypass,
                replica_groups=[list(range(NC))],
                ins=[b_in.opt()], outs=[b_out.opt()])
            roots_src = b_out

        if part == "p2" or (part == "all" and stop_after is None):
            # ---- top tree (levels 2..0) from the gathered roots ----
            if part == "p2":
                roots_src = roots_d
                latT = LATALL[:, 0:T]
            else:
                latT = LATALL[:, _TOP_OFF:_TOP_OFF + T]
            rtA = sc("s5", 128)
            nc.sync.dma_start(rtA[0:4, :], roots_src[0:4, :])
            rtB = sc("s6", 128)
            nc.sync.dma_start(rtB[0:4, :], roots_src[4:8, :])
            nc.vector.tensor_add(latT[0:4, :], latT[0:4, :], rtA[0:4, :])
            nc.vector.tensor_add(latT[0:4, :], latT[0:4, :], rtB[0:4, :])

            Itp = spool.tile([128, T + 1], f32, tag="bI")
            nc.vector.memset(Itp[0:R_TOP, 0:1], 0.0)
            Otp = opool.tile([128, T + 1], f32, tag="bigO")
            nc.gpsimd.memset(Otp[0:R_TOP, 1:], 0.0)
            emit_group(R_TOP, S_TOP, NBIG_TILES + 1, Otp, Itp[:], 0,
                       latE=latT, lhsT=amat[0:R_TOP, R_STACK:])
            nc.sync.dma_start(out_d, Otp[R_TOP - 1:R_TOP, 1:])

    nc.compile()
    return nc


def kernel(**inputs):
    from concourse.bass_utils import run_bass_kernel_spmd

    in_maps = _host_precompute(inputs)
    dtf = float(inputs["dt"])
    nc1 = _build_bass(dtf, part="p1")
    res1 = run_bass_kernel_spmd(nc1, in_maps, core_ids=list(range(NC)))
    roots = np.ascontiguousarray(
        np.stack([np.asarray(res1.results[c]["root"]).reshape(-1)
                  for c in range(NC)]).astype(F32))
    nc2 = _build_bass(dtf, part="p2")
    in_maps2 = [dict(m, roots=roots) for m in in_maps]
    res2 = run_bass_kernel_spmd(nc2, in_maps2, core_ids=list(range(NC)))
    out = res2.results[0]["out"].reshape(-1)
    return out.astype(F32)


if __name__ == "__main__":
    data = np.load("/root/problem/inputs_cache.npz")
    inputs = {k: data[k] for k in data.files}
    out = kernel(**inputs)
    exp = np.load("/root/problem/expected.npy")
    err = np.abs(out - exp) / (np.abs(exp) + 1e-6)
    print("kernel[:4]", out[:4], "expected[:4]", exp[:4])
    print("max rel err", err.max())


# revision 24
# speedup vs baseline: 1.0539x; 1.0539x over previous
"""Muskingum-Cunge river routing (depth-13 binary tree, N=8191, T=2048) on
8 Trainium2 NeuronCores — parallel-in-time Picard solver, v3.

Per reach, the MC update O_t = C1 I_t + C2 I_{t-1} + C3 O_{t-1} is a linear
recurrence once the (flow-dependent) coefficients are frozen; each Picard
pass recomputes the coefficients from the previous trajectory and re-solves
the recurrence with one DVE tensor_tensor_scan along time (the scan is
serial along the free dim, ~9 cycles/column, so scan instances are the
budget that matters). The relu clamp is folded into the fixed point via
B = (D + r) - q2*((K/dt)D + (r - I_old)) with r = relu(-U_prev).

Structure per core (core c owns the subtree under the c-th level-3 node):
  - levels 12..10 (512/256/128 nodes): node-major [128, T] tiles,
    K=(1,2,2) Picard passes, exact child inputs.
  - levels 9..3 (127 nodes): ONE stacked [127, T] Jacobi-Picard system,
    S_STACK passes; parent<-children pair-sum via a 0/1 matmul on the idle
    PE. No chunked layout, no DRAM scatter/gather, no carry machinery.
  - AllGather of the 8 level-3 root trajectories (64 KB), then the 7-node
    top tree (levels 2..0) as another stacked system, S_TOP passes.
All lateral inflows arrive in one partition-major [128, ~18.4k] block so
the whole 8.4 MB loads with three fat DMAs (384 descriptors, each under
the 64 KB descriptor limit) instead of ~1100 row-tile descriptors.

kernel() runs the split p1/p2 programs (host-gathered roots, no on-device
collective); the timed path uses part="all" (one NEFF per execution, the
AllGather in-NEFF) because every extra NEFF launch in a pipelined chain
costs ~3 ms on this runtime.
"""
import os
import sys

import numpy as np

for _p in ("/opt/trn_rl_repo", "/root/.axon_site/_ro/trn_rl_repo"):
    if os.path.isdir(_p) and _p not in sys.path:
        sys.path.insert(0, _p)

DEPTH = 13
N = 2**DEPTH - 1
T = 2048
NC = 8
F32 = np.float32

K_BIG = (1, 2, 2)  # Picard passes for levels 12, 11, 10
S_STACK = 8        # joint Jacobi-Picard passes for stacked levels 9..3
S_TOP = 4          # joint passes for the top tree (levels 2..0)

BIG_LEVELS = [(12, 512), (11, 256), (10, 128)]
K_OF_LEVEL = {12: K_BIG[0], 11: K_BIG[1], 10: K_BIG[2]}
STACK_LEVELS = [9, 8, 7, 6, 5, 4, 3]           # 64+32+16+8+4+2+1 = 127 rows
R_STACK = 127
R_TOP = 7
NBIG_TILES = 4 + 2 + 1
NGRP = NBIG_TILES + 2                          # const col groups: 7 big + stack + top
NCOL = NGRP * 4 + 1
_LN2DT_COL = NGRP * 4
AMAT_COLS = R_STACK + R_TOP                    # stack lhsT (127) + top lhsT (7)

# LAT2 layout: 7 big-tile blocks of 2049 cols (col 0 zero), then the stack
# block (2048 cols, rows 0..126), then the top block (2048 cols, rows 0..6).
_BIG_OFF = [g * 2049 for g in range(NBIG_TILES)]
_STACK_OFF = NBIG_TILES * 2049
_TOP_OFF = _STACK_OFF + T
LAT2_COLS = _TOP_OFF + T


def _build_ord():
    ORD = [np.array([0], dtype=np.int64)]
    for l in range(DEPTH - 1):
        cur = ORD[l]
        nxt = np.empty(2 * len(cur), dtype=np.int64)
        nxt[: len(cur)] = 2 * cur + 1
        nxt[len(cur):] = 2 * cur + 2
        ORD.append(nxt)
    return ORD


ORD = _build_ord()


def _level_nodes(core, lv):
    return ORD[lv] if lv < 3 else ORD[lv][core::NC]


# stack row base per level (level 9 first)
_STACK_BASE = {}
_off = 0
for _lv in STACK_LEVELS:
    _STACK_BASE[_lv] = _off
    _off += len(ORD[_lv]) // NC


def _stack_amat_T():
    """[127, 127] lhsT: A_T[child_row, parent_row] = 1 (in-stack coupling)."""
    A = np.zeros((R_STACK, R_STACK), F32)
    for lv in STACK_LEVELS[1:]:                 # parents: levels 8..3
        nl = len(ORD[lv]) // NC
        bp = _STACK_BASE[lv]
        bc = _STACK_BASE[lv + 1]
        for j in range(nl):
            A[bc + j, bp + j] = 1.0
            A[bc + j + nl, bp + j] = 1.0
    return A


def _top_amat_T():
    """[7, 7] lhsT for top rows: 0..3 = level-2 canon (ORD[2]), 4..5 =
    level-1 canon (ORD[1]), 6 = root."""
    A = np.zeros((R_TOP, R_TOP), F32)
    # level-1 canon j (rows 4,5): children = level-2 canon j and j+2
    A[0, 4] = A[2, 4] = 1.0
    A[1, 5] = A[3, 5] = 1.0
    # root (row 6): children = level-1 canon 0,1
    A[4, 6] = A[5, 6] = 1.0
    return A


def _host_precompute(inputs):
    lat = np.ascontiguousarray(np.asarray(inputs["lateral_inflows"], F32))
    n_ = np.asarray(inputs["manning_n"], F32).astype(np.float64)
    L = np.asarray(inputs["lengths"], F32).astype(np.float64)
    S = np.asarray(inputs["slopes"], F32).astype(np.float64)
    wc = np.asarray(inputs["width_coefs"], F32).astype(np.float64)
    we = np.asarray(inputs["width_exps"], F32).astype(np.float64)
    dc = np.asarray(inputs["depth_coefs"], F32).astype(np.float64)
    de = np.asarray(inputs["depth_exps"], F32).astype(np.float64)
    c0 = (5.0 / 3.0) * dc ** (2.0 / 3.0) * np.sqrt(S) / n_
    a1n = -(2.0 / 3.0) * de
    a3 = 1.0 - we - (2.0 / 3.0) * de
    ln_half = np.log(0.5)
    P4 = np.log(L / c0) + a1n * ln_half
    P3 = np.log(0.5 / (wc * S * L * c0)) + a3 * ln_half + np.log(2.0)
    consts = np.stack([a1n, a3, P4, P3]).astype(F32)      # [4, N]

    amat = np.zeros((128, AMAT_COLS), F32)
    amat[:R_STACK, :R_STACK] = _stack_amat_T()
    amat[:R_TOP, R_STACK:] = _top_amat_T()

    top_nodes = np.concatenate([ORD[2], ORD[1], ORD[0]])

    in_maps = []
    for core in range(NC):
        LAT2 = np.zeros((128, LAT2_COLS), F32)
        stack_nodes = np.concatenate(
            [_level_nodes(core, lv) for lv in STACK_LEVELS])
        g = 0
        for lv, n in BIG_LEVELS:
            nodes = _level_nodes(core, lv)
            for ti in range(n // 128):
                LAT2[:, _BIG_OFF[g] + 1:_BIG_OFF[g] + 1 + T] = \
                    lat[:, nodes[ti * 128:(ti + 1) * 128]].T
                g += 1
        LAT2[:R_STACK, _STACK_OFF:_STACK_OFF + T] = lat[:, stack_nodes].T
        LAT2[:R_TOP, _TOP_OFF:_TOP_OFF + T] = lat[:, top_nodes].T

        ccols = []
        for lv, n in BIG_LEVELS:
            nodes = _level_nodes(core, lv)
            for ti in range(n // 128):
                ccols.append(consts[:, nodes[ti * 128:(ti + 1) * 128]])
        cs = np.zeros((4, 128), F32)
        cs[:, :R_STACK] = consts[:, stack_nodes]
        ccols.append(cs)
        ct = np.zeros((4, 128), F32)
        ct[:, :R_TOP] = consts[:, top_nodes]
        ccols.append(ct)
        ccols.append(np.full((1, 128), np.log(2.0 * float(inputs["dt"])), F32))
        CST = np.ascontiguousarray(np.concatenate(ccols, axis=0).T.astype(F32))
        in_maps.append({"lat": np.ascontiguousarray(LAT2), "cst": CST,
                        "amat": amat})
    return in_maps


def _build_bass(dtf, part="all", stop_after=None):
    """Build the bass program.

    part="all": the whole routing step in one program, with the in-NEFF
        AllGather of root trajectories (used by the timed path: one NEFF
        launch per execution).
    part="p1": big + stacked levels only; outputs this core's level-3 root
        trajectory `root` [1, T]. No collectives.
    part="p2": inputs the 8 gathered root trajectories `roots` [8, T]
        (core/canon order); solves the top tree, outputs `out` [1, T].
    """
    from contextlib import ExitStack

    import concourse.bass as bass
    import concourse.tile as tile
    from concourse import bacc, mybir

    f32 = mybir.dt.float32
    OP = mybir.AluOpType
    AF = mybir.ActivationFunctionType
    inv_dt = 1.0 / dtf

    nc = bacc.Bacc("TRN2", target_bir_lowering=False, debug=False,
                   num_devices=NC)
    lat_d = nc.dram_tensor("lat", [128, LAT2_COLS], f32,
                           kind="ExternalInput").ap()
    cst_d = nc.dram_tensor("cst", [128, NCOL], f32, kind="ExternalInput").ap()
    amat_d = nc.dram_tensor("amat", [128, AMAT_COLS], f32,
                            kind="ExternalInput").ap()
    if part == "p1":
        out_d = nc.dram_tensor("root", [1, T], f32, kind="ExternalOutput").ap()
    elif part == "p2":
        roots_d = nc.dram_tensor("roots", [NC, T], f32,
                                 kind="ExternalInput").ap()
        out_d = nc.dram_tensor("out", [1, T], f32, kind="ExternalOutput").ap()
    else:
        out_d = nc.dram_tensor("out", [1, T], f32, kind="ExternalOutput").ap()

    with tile.TileContext(nc) as tc, ExitStack() as ctx:
        cpool = ctx.enter_context(tc.tile_pool(name="const", bufs=1))
        opool = ctx.enter_context(tc.tile_pool(name="lvlO", bufs=5))
        spool = ctx.enter_context(tc.tile_pool(name="scr", bufs=1))
        psum = ctx.enter_context(tc.tile_pool(name="ps", bufs=4, space="PSUM"))
        if part == "all":
            dram = ctx.enter_context(tc.tile_pool(name="dram", bufs=1,
                                                  space="DRAM"))

        cst = cpool.tile([128, NCOL], f32)
        nc.sync.dma_start(cst[:], cst_d)
        amat = cpool.tile([128, AMAT_COLS], f32)
        nc.sync.dma_start(amat[:], amat_d)

        if part in ("all", "p1"):
            LATALL = spool.tile([128, LAT2_COLS], f32, tag="LAT")
            # chunked: one descriptor per partition per chunk must stay
            # under the 64 KB DMA descriptor size limit
            third = 6150
            for c0 in range(0, LAT2_COLS, third):
                c1 = min(c0 + third, LAT2_COLS)
                nc.sync.dma_start(LATALL[:, c0:c1], lat_d[:, c0:c1])
        else:
            LATALL = spool.tile([128, T], f32, tag="LAT")
            nc.sync.dma_start(LATALL[:], lat_d[:, _TOP_OFF:_TOP_OFF + T])

        def cc(grp, R):
            c0 = grp * 4
            return (cst[0:R, c0:c0 + 1], cst[0:R, c0 + 1:c0 + 2],
                    cst[0:R, c0 + 2:c0 + 3], cst[0:R, c0 + 3:c0 + 4])

        def sc(tag, R):
            t = spool.tile([128, T], f32, tag=tag, name=f"scr_{tag}")
            return t[0:R, :]

        def emit_group(R, S, grp, Obuf, Ibase, icol, latE=None, lhsT=None):
            """Picard-solve one group of R reaches over the full horizon.

            The inflow lives in Ibase[0:R, icol:icol+T+1]; col icol must be
            zero. If latE is None the inflow is fixed (big levels); else each
            pass rebuilds it as latE + lhsT.T @ O and Obuf[:,1:] must start
            zeroed. Obuf: [128, T+1]; on return holds O with col 0 = 0.
            """
            a1, a3, P4, P3 = cc(grp, R)
            U = spool.tile([128, T + 1], f32, tag="gU")
            r = spool.tile([128, T + 1], f32, tag="gr")
            nc.vector.memset(r[0:R, :], 0.0)
            nc.vector.memset(Obuf[0:R, 0:1], 0.0)
            I1 = Ibase[0:R, icol + 1:icol + 1 + T]
            I0 = Ibase[0:R, icol:icol + T]
            for it in range(S):
                if latE is not None:
                    for j in range(4):
                        ps = psum.tile([128, 512], f32, tag=f"ps{j % 2}")
                        nc.tensor.matmul(ps[0:R, :], lhsT,
                                         Obuf[0:R, 1 + 512 * j:1 + 512 * (j + 1)],
                                         start=True, stop=True)
                        nc.vector.tensor_add(
                            Ibase[0:R, icol + 1 + 512 * j:icol + 1 + 512 * (j + 1)],
                            ps[0:R, :], latE[0:R, 512 * j:512 * (j + 1)])
                Oold = I0 if it == 0 else Obuf[0:R, 0:T]
                s1 = sc("s1", R)
                nc.vector.tensor_add(s1, I1, Oold)
                s2 = sc("s2", R)
                nc.vector.tensor_scalar(s2, s1, 2e-3, None, op0=OP.max)
                lq = sc("s1", R)
                nc.scalar.activation(lq, s2, AF.Ln)
                K_ = sc("s3", R)
                nc.scalar.activation(K_, lq, AF.Exp, bias=P4, scale=a1)
                tt = sc("s2", R)
                nc.scalar.activation(tt, lq, AF.Exp, bias=P3, scale=a3)
                w1 = sc("s4", R)
                nc.vector.tensor_scalar(w1, tt, 1.0, None, op0=OP.min)
                v1 = sc("s2", R)
                nc.vector.tensor_mul(v1, K_, w1)
                den = sc("s4", R)
                nc.vector.scalar_tensor_tensor(den, K_, dtf, v1, OP.add,
                                               OP.add)
                lnd = sc("s8", R)
                nc.scalar.activation(lnd, den, AF.Ln)
                q2 = sc("s2", R)
                nc.scalar.activation(q2, lnd, AF.Exp,
                                     bias=cst[0:R, _LN2DT_COL:_LN2DT_COL + 1],
                                     scale=-1.0)
                C3 = sc("s4", R)
                nc.vector.tensor_scalar(C3, q2, -1.0, 1.0, op0=OP.mult,
                                        op1=OP.add)
                D = sc("s8", R)
                nc.vector.tensor_sub(D, I1, I0)
                E = sc("s5", R)
                nc.gpsimd.tensor_add(E, D, r[0:R, 0:T])
                G = sc("s6", R)
                nc.gpsimd.tensor_sub(G, r[0:R, 0:T], I0)
                h1 = sc("s7", R)
                nc.gpsimd.tensor_mul(h1, K_, D)
                H = sc("s3", R)
                nc.vector.scalar_tensor_tensor(H, h1, inv_dt, G, OP.mult,
                                               OP.add)
                zz = sc("s7", R)
                nc.vector.tensor_mul(zz, q2, H)
                B = sc("s1", R)
                nc.vector.tensor_sub(B, E, zz)
                nc.vector.tensor_tensor_scan(U[0:R, 1:], C3, B, 0.0,
                                             OP.mult, OP.add)
                nc.scalar.activation(Obuf[0:R, 1:], U[0:R, 1:], AF.Relu)
                if it < S - 1:
                    nc.scalar.activation(r[0:R, 1:], U[0:R, 1:], AF.Relu,
                                         scale=-1.0)

        Ost = None
        if part in ("all", "p1"):
            # ---- big levels (12, 11, 10): exact inputs, fixed inflow ----
            grp = 0
            child_tiles = None
            O10 = None
            for lv, n in BIG_LEVELS:
                ntile = n // 128
                tiles = []
                for ti in range(ntile):
                    off = _BIG_OFF[grp]
                    if lv != 12:
                        sup = sc("s6", 128)
                        nc.vector.tensor_add(sup, child_tiles[ti][:, 1:],
                                             child_tiles[ti + ntile][:, 1:])
                        nc.vector.tensor_add(
                            LATALL[:, off + 1:off + 1 + T],
                            LATALL[:, off + 1:off + 1 + T], sup)
                    Obig = opool.tile([128, T + 1], f32, tag="bigO")
                    emit_group(128, K_OF_LEVEL[lv], grp, Obig, LATALL[:], off)
                    tiles.append(Obig)
                    grp += 1
                child_tiles = tiles
                if lv == 10:
                    O10 = tiles[0]

            if stop_after == "big":
                nc.sync.dma_start(out_d, O10[0:1, 1:])
            else:
                # ---- stacked levels 9..3 ----
                tmp = sc("s5", 128)
                nc.sync.dma_start(tmp[0:64, :], O10[64:128, 1:])
                latE = LATALL[:, _STACK_OFF:_STACK_OFF + T]
                nc.vector.tensor_add(latE[0:64, :], latE[0:64, :],
                                     O10[0:64, 1:])
                nc.vector.tensor_add(latE[0:64, :], latE[0:64, :],
                                     tmp[0:64, :])

                Ist = spool.tile([128, T + 1], f32, tag="bI")
                nc.vector.memset(Ist[0:R_STACK, 0:1], 0.0)
                Ost = opool.tile([128, T + 1], f32, tag="bigO")
                nc.gpsimd.memset(Ost[0:R_STACK, 1:], 0.0)
                emit_group(R_STACK, S_STACK, NBIG_TILES, Ost, Ist[:], 0,
                           latE=latE, lhsT=amat[0:R_STACK, 0:R_STACK])
                if part == "p1":
                    nc.sync.dma_start(out_d, Ost[R_STACK - 1:R_STACK, 1:])

        if part == "all" and stop_after is None:
            b_in = dram.tile([1, T], f32)
            b_out = dram.tile([NC, T], f32)
            nc.sync.dma_start(b_in[:], Ost[R_STACK - 1:R_STACK, 1:])
            nc.gpsimd.collective_compute(
                "AllGather", OP.bypass,
                replica_groups=[list(range(NC))],
                ins=[b_in.opt()], outs=[b_out.opt()])
            roots_src = b_out

        if part == "p2" or (part == "all" and stop_after is None):
            # ---- top tree (levels 2..0) from the gathered roots ----
            if part == "p2":
                roots_src = roots_d
                latT = LATALL[:, 0:T]
            else:
                latT = LATALL[:, _TOP_OFF:_TOP_OFF + T]
            rtA = sc("s5", 128)
            nc.sync.dma_start(rtA[0:4, :], roots_src[0:4, :])
            rtB = sc("s6", 128)
            nc.sync.dma_start(rtB[0:4, :], roots_src[4:8, :])
            nc.vector.tensor_add(latT[0:4, :], latT[0:4, :], rtA[0:4, :])
            nc.vector.tensor_add(latT[0:4, :], latT[0:4, :], rtB[0:4, :])

            Itp = spool.tile([128, T + 1], f32, tag="bI")
            nc.vector.memset(Itp[0:R_TOP, 0:1], 0.0)
            Otp = opool.tile([128, T + 1], f32, tag="bigO")
            nc.gpsimd.memset(Otp[0:R_TOP, 1:], 0.0)
            emit_group(R_TOP, S_TOP, NBIG_TILES + 1, Otp, Itp[:], 0,
                       latE=latT, lhsT=amat[0:R_TOP, R_STACK:])
            nc.sync.dma_start(out_d, Otp[R_TOP - 1:R_TOP, 1:])

    nc.compile()
    return nc


def kernel(**inputs):
    from concourse.bass_utils import run_bass_kernel_spmd

    in_maps = _host_precompute(inputs)
    dtf = float(inputs["dt"])
    nc1 = _build_bass(dtf, part="p1")
    res1 = run_bass_kernel_spmd(nc1, in_maps, core_ids=list(range(NC)))
    roots = np.ascontiguousarray(
        np.stack([np.asarray(res1.results[c]["root"]).reshape(-1)
                  for c in range(NC)]).astype(F32))
    nc2 = _build_bass(dtf, part="p2")
    in_maps2 = [dict(m, roots=roots) for m in in_maps]
    res2 = run_bass_kernel_spmd(nc2, in_maps2, core_ids=list(range(NC)))
    out = res2.results[0]["out"].reshape(-1)
    return out.astype(F32)


if __name__ == "__main__":
    data = np.load("/root/problem/inputs_cache.npz")
    inputs = {k: data[k] for k in data.files}
    out = kernel(**inputs)
    exp = np.load("/root/problem/expected.npy")
    err = np.abs(out - exp) / (np.abs(exp) + 1e-6)
    print("kernel[:4]", out[:4], "expected[:4]", exp[:4])
    print("max rel err", err.max())
